# revision 39
# baseline (speedup 1.0000x reference)
"""BinaryLinear kernel for Trainium2 (8 NeuronCores, SPMD).

Computes  out = sign(x) @ sign(W)^T * alpha  for
x: [8192, 2048] f32, W: [2048, 2048] f32, alpha: [1] f32.

Strategy: data-parallel over the token dim (8 shards of 1024 tokens);
W replicated. The op only reads the sign of each input element, so the
host ships just the MSB byte of every f32 (sign + exponent bits — a
pure byte-slice, all arithmetic stays on device): x 2 MiB/core, W
4 MiB/core instead of 8+16 MiB. On device a single DVE op per chunk
maps 4 packed sign-bytes at a time to fp8(E4M3) +-1 via
(b & 0x80) | 0x38 on u32 bitcast views (+-1 is exact; accumulation of
<=2048 +-1 terms is exact in fp32 PSUM). DoubleRow fp8 matmuls (2
k-tiles per MM) then run back-to-back; PSUM drains scale by alpha and
write fp16 (all outputs are small even integers — exact), halving
output traffic.

Rings: sync carries W chunks, gpsimd carries alpha + x chunks, scalar
carries output writes. DVE does all sign ops; drains alternate
DVE/ACT. The first matmul can start ~1.5us in, and the tensor engine
(the true bottleneck at ~222ns per DoubleRow MM) stays saturated.
"""

import numpy as np

import concourse.bass as bass
import concourse.tile as tile
from concourse import bacc, mybir
from concourse.bass_utils import run_bass_kernel_spmd

N_CORES = 8
NTOK = 8192
INF = 2048
OUTF = 2048
TPC = NTOK // N_CORES  # tokens per core (1024)
P = 128
KT = INF // P  # 16 contraction tiles
MT = TPC // P  # 8 token tiles per core
NTS = 512  # out_features per matmul (one PSUM bank)
NT = OUTF // NTS  # 4

F32 = mybir.dt.float32
F16 = mybir.dt.float16
FP8 = mybir.dt.float8e4  # E4M3; +-1.0 is exact
U8 = mybir.dt.uint8
U32 = mybir.dt.uint32

MASK_AND = 0x80808080  # keep sign bit of each packed byte
MASK_OR = 0x38383838  # set exponent bits of +-1.0 in E4M3

# k-tile chunk schedule. DMA cost ~ 1.4us fixed + bytes/rate where the
# rate grows with per-partition run length (~75 GB/s @1KB runs, ~230
# @4KB, ~410 @8KB), so chunks are 4kt x (4KB runs) and 8-16kt W.
# Queues: scalar carries the two head x chunks (then output), gpsimd
# (slowest) the two tail x chunks, sync all of W.
X_CHUNKS = [4, 4, 4, 4]  # kt per chunk, packed sequentially
# W chunks (n, k0, sz): n0 split as kt0-1 (tiny head chunk so the first
# matmul starts early), kt2-7, kt8-15; n1..n3 whole.
W_DEFS = [(0, 0, 2), (0, 2, 6), (0, 8, 8), (1, 0, 16), (2, 0, 16),
          (3, 0, 16)]
N_DUMMY_MM = 40  # small warm-up matmuls to hold the PE clock at 2.4GHz

_compiled = None
LAST_RESULT = None  # BassKernelResults of the most recent run (for profiling)


def _build():
    nc = bacc.Bacc(
        "TRN2",
        target_bir_lowering=False,
        debug=False,
        num_devices=N_CORES,
    )
    xb = nc.dram_tensor("xb", [P * KT * TPC], U8, kind="ExternalInput").ap()
    wb = nc.dram_tensor("wb", [P * NT * KT * NTS], U8, kind="ExternalInput").ap()
    al = nc.dram_tensor("alpha", [P, 1], F32, kind="ExternalInput").ap()
    out = nc.dram_tensor(
        "out", [NT, MT // 2, P, 2 * NTS], F16, kind="ExternalOutput"
    ).ap()

    with tile.TileContext(nc) as tc:
        with (
            tc.tile_pool(name="res", bufs=1) as res,
            tc.tile_pool(name="wlate", bufs=1) as wlate,
            tc.tile_pool(name="psum", bufs=8, space="PSUM") as ppool,
            tc.tile_pool(name="outp", bufs=2) as outp,
        ):
            # One tile per DMA chunk: the tile dep tracker coarsens
            # adjacent writes within a tile, so shared tiles create
            # false waits (e.g. the first matmul waiting on a late
            # chunk's sign op). Separate tiles make deps exact.
            XC = len(X_CHUNKS)  # 4 chunks of 4 k-tiles
            xraws = [res.tile([P, 4, TPC], U8, name=f"xraw{i}")
                     for i in range(XC)]
            bxs = [res.tile([P, 4, TPC], FP8, name=f"bx{i}")
                   for i in range(XC)]
            # W1/W2/W3 raw staging cycles through ONE wlate buffer
            # (bufs=1, same tag): W2's DMA then genuinely depends on
            # W1's sign having consumed the buffer (and W3 on W2's),
            # which keeps those 2MB out of the oversubscribed early DMA
            # window — a real dependency the scheduler cannot hoist
            # past. Tiles are allocated at their load site so the
            # reuse order matches the instruction stream.
            wraws = {(n, k0): res.tile([P, sz, NTS], U8,
                                       name=f"wraw{n}_{k0}")
                     for n, k0, sz in W_DEFS if n == 0}
            bws = {(n, k0): res.tile([P, sz, NTS], FP8,
                                     name=f"bw{n}_{k0}")
                   for n, k0, sz in W_DEFS}
            alpha_t = res.tile([P, 1], F32)

            def sign_op(dst, src):
                nc.vector.tensor_scalar(
                    dst.bitcast(U32),
                    src.bitcast(U32),
                    MASK_AND,
                    MASK_OR,
                    op0=mybir.AluOpType.bitwise_and,
                    op1=mybir.AluOpType.bitwise_or,
                )

            # Warm-up: tiny matmuls on a zeroed tile keep the PE HAM
            # activity monitor busy through the DMA fill so the real
            # matmuls run at 2.4GHz from the start.
            dummy = res.tile([P, 2, P], FP8)
            psd = ppool.tile([P, NTS], F32, name="ps", tag="ps")
            nc.gpsimd.memset(dummy[:], 0)
            for _ in range(N_DUMMY_MM):
                nc.tensor.matmul(
                    psd[:, 0:P],
                    dummy[:],
                    dummy[:],
                    start=True,
                    stop=True,
                    perf_mode=mybir.MatmulPerfMode.DoubleRow,
                )

            x_off = [0]

            def load_x_chunk(ci, engine):
                nbytes = P * 4 * TPC
                flat = xb[x_off[0] : x_off[0] + nbytes]
                dst = xraws[ci][:].rearrange("p a b -> p (a b)")
                engine.dma_start(dst, flat.rearrange("(p f) -> p f", p=P))
                x_off[0] += nbytes

            def sign_x_chunk(ci):
                sign_op(
                    bxs[ci][:].rearrange("p a b -> p (a b)"),
                    xraws[ci][:].rearrange("p a b -> p (a b)"),
                )

            w_off = [0]

            def load_w_chunk(n, k0, sz, engine):
                if n >= 1:
                    wraws[(n, k0)] = wlate.tile(
                        [P, KT, NTS], U8, name="wl", tag="wl"
                    )
                flat = wb[w_off[0] : w_off[0] + P * sz * NTS]
                dst = wraws[(n, k0)][:].rearrange("p a b -> p (a b)")
                engine.dma_start(dst, flat.rearrange("(p f) -> p f", p=P))
                w_off[0] += P * sz * NTS

            def sign_w_chunk(n, k0):
                sign_op(
                    bws[(n, k0)][:].rearrange("p a b -> p (a b)"),
                    wraws[(n, k0)][:].rearrange("p a b -> p (a b)"),
                )

            # ---- load + sign phase ----
            # The three DMA queues share ~370 GB/s aggregate in the
            # early window, so only chunks needed before ~25us are
            # issued up front; W2/W3 DMAs are deferred into the matmul
            # phase. Queue plan:
            #   sync:   x kt0-3, x kt4-7, W0 kt8-15, W1
            #   scalar: W0 kt0-1, W0 kt2-7, x kt8-11, (W2/W3 + output
            #           later)
            #   gpsimd: alpha, x kt12-15
            # DVE sign ops follow in expected arrival order.
            load_x_chunk(0, nc.sync)
            load_w_chunk(0, 0, 2, nc.scalar)  # w0 head: kt0-1
            nc.gpsimd.dma_start(alpha_t[:], al)
            load_w_chunk(0, 2, 6, nc.scalar)
            load_x_chunk(1, nc.sync)
            load_x_chunk(2, nc.scalar)
            load_x_chunk(3, nc.gpsimd)
            load_w_chunk(0, 8, 8, nc.sync)
            load_w_chunk(1, 0, 16, nc.sync)

            sign_x_chunk(0)
            sign_w_chunk(0, 0)
            sign_w_chunk(0, 2)
            sign_x_chunk(1)
            sign_x_chunk(2)
            sign_w_chunk(0, 8)
            sign_x_chunk(3)
            sign_w_chunk(1, 0)

            def mm(ps_ap, m, n, k):
                if n == 0:
                    wk0 = 0 if k < 2 else (2 if k < 8 else 8)
                    rhs = bws[(0, wk0)][:, k - wk0 : k - wk0 + 2, :]
                else:
                    rhs = bws[(n, 0)][:, k : k + 2, :]
                ci = k // 4
                lhsT = bxs[ci][:, k % 4 : k % 4 + 2, m * P : (m + 1) * P]
                nc.tensor.matmul(
                    ps_ap,
                    lhsT,
                    rhs,
                    start=(k == 0),
                    stop=(k + 2 >= KT),
                    perf_mode=mybir.MatmulPerfMode.DoubleRow,
                )

            def drain(dst, ps, idx):
                # Alternate DVE/ACT so consecutive drains run in parallel.
                if idx % 2 == 0:
                    nc.scalar.activation(
                        dst, ps, mybir.ActivationFunctionType.Copy,
                        scale=alpha_t[:],
                    )
                else:
                    nc.vector.tensor_scalar_mul(dst, ps, alpha_t[:])

            def drain_and_store(obuf, pss, n):
                for m in range(MT):
                    drain(obuf[:, m, :], pss[m][:], m)
                    if m % 2 == 1:
                        nc.scalar.dma_start(
                            out[n, m // 2],
                            obuf[:, m - 1 : m + 1, :].rearrange(
                                "p a b -> p (a b)"
                            ),
                        )

            # ---- matmul phase ----
            # n=0: k-middle / m-inner so matmuls start on the first k-pair.
            obuf = outp.tile([P, MT, NTS], F16)
            pss = [
                ppool.tile([P, NTS], F32, name="ps", tag="ps")
                for _ in range(MT)
            ]
            for k in range(0, KT, 2):
                for m in range(MT):
                    mm(pss[m][:], m, 0, k)
            drain_and_store(obuf, pss, 0)

            # Deferred W2 load: the shared wlate buffer makes its DMA
            # wait for W1's sign, keeping the early DMA window free.
            load_w_chunk(2, 0, 16, nc.sync)
            sign_w_chunk(2, 0)

            # n=1..3: m-outer / k-inner; drain overlaps the next m's MMs.
            for n in range(1, NT):
                obuf = outp.tile([P, MT, NTS], F16)
                pss = []
                for m in range(MT):
                    ps = ppool.tile([P, NTS], F32, name="ps", tag="ps")
                    for k in range(0, KT, 2):
                        mm(ps[:], m, n, k)
                    pss.append(ps)
                    if n == NT - 1 and m == MT - 1:
                        # Last drain: halves on ACT+DVE concurrently so
                        # the final out-DMA starts as early as possible.
                        h = NTS // 2
                        nc.scalar.activation(
                            obuf[:, m, 0:h], ps[:, 0:h],
                            mybir.ActivationFunctionType.Copy,
                            scale=alpha_t[:],
                        )
                        nc.vector.tensor_scalar_mul(
                            obuf[:, m, h:NTS], ps[:, h:NTS], alpha_t[:]
                        )
                    else:
                        drain(obuf[:, m, :], ps[:], m)
                    if m % 2 == 1:
                        # Final pair goes on the (idle) sync queue so it
                        # isn't serialized behind earlier output DMAs.
                        eng = (
                            nc.sync
                            if n == NT - 1 and m == MT - 1
                            else nc.scalar
                        )
                        eng.dma_start(
                            out[n, m // 2],
                            obuf[:, m - 1 : m + 1, :].rearrange(
                                "p a b -> p (a b)"
                            ),
                        )
                if n == 1:
                    # Deferred W3 load (waits on W2's sign via wlate).
                    load_w_chunk(3, 0, 16, nc.sync)
                    sign_w_chunk(3, 0)

    nc.compile()
    return nc


def _msb(a):
    # MSB byte of each little-endian f32: sign bit + top exponent bits.
    return a.view(np.uint8).reshape(a.shape[0], a.shape[1], 4)[:, :, 3]


def _pack_w(weight):
    # W^T[k, o] MSB bytes -> chunks of [P, sz, NTS] in DMA issue order.
    w4 = _msb(weight).T.reshape(KT, P, NT, NTS)
    parts = []
    for n, k0, sz in W_DEFS:
        parts.append(w4[k0 : k0 + sz, :, n, :].transpose(1, 0, 2).ravel())
    return np.ascontiguousarray(np.concatenate(parts))


def _pack_x_shard(xs):
    # xs: [TPC, INF] MSB bytes -> chunks of [P, sz, TPC] in DMA issue order.
    x4 = _msb(xs).T.reshape(KT, P, TPC)
    parts = []
    k0 = 0
    for sz in X_CHUNKS:
        parts.append(x4[k0 : k0 + sz].transpose(1, 0, 2).ravel())
        k0 += sz
    return np.ascontiguousarray(np.concatenate(parts))


def kernel(x, weight, alpha):
    global _compiled, LAST_RESULT
    if _compiled is None:
        _compiled = _build()
    nc = _compiled

    x = np.asarray(x, dtype=np.float32)
    weight = np.asarray(weight, dtype=np.float32)
    alpha = np.asarray(alpha, dtype=np.float32)

    wpk = _pack_w(weight)
    alv = np.full((P, 1), alpha.reshape(-1)[0], dtype=np.float32)
    in_maps = []
    for c in range(N_CORES):
        xs = _pack_x_shard(x[c * TPC : (c + 1) * TPC, :])
        in_maps.append({"xb": xs, "wb": wpk, "alpha": alv})

    LAST_RESULT = run_bass_kernel_spmd(nc, in_maps, list(range(N_CORES)))
    outs = []
    for c in range(N_CORES):
        o = LAST_RESULT.results[c]["out"]  # [NT, MT//2, P, 2*NTS] f16
        o = o.reshape(NT, MT // 2, P, 2, NTS).astype(np.float32)
        # -> [MT//2, 2, P, NT, NTS] -> [TPC, OUTF]
        outs.append(o.transpose(1, 3, 2, 0, 4).reshape(TPC, OUTF))
    return np.concatenate(outs, axis=0)
